# revision 20
# baseline (speedup 1.0000x reference)
"""Causal self-attention (B=4, T=2048, C=1024, 16 heads) on 8 TRN2 NeuronCores.

Sharding: data-parallel over batch (4) x tensor-parallel over heads (2 groups
of 8). Each core computes qkv + attention for its 8 heads and a partial
output projection (row-parallel); the host sums the two partials per batch.

Everything on-chip runs in a transposed layout so no tensor ever needs an
on-device transpose:
  QT/KT [ch, t]  <- W^T @ x^T      (x^T prepared on host)
  attT  [k, q]   = exp(K @ Q^T / 8) * causal_mask
  yT    [ch, q]  = V_aug^T @ attT  (V augmented with a ones column per head ->
                                    row 64 of each head's block = softmax denom)
  out   [q, c]   = yT^T @ Wp       (partial; host-reduced across head groups)

Softmax skips max-subtraction (scores/8 are O(1) here, exp is safe), which is
mathematically identical to the reference; fully-masked blocks are never
computed, straddle blocks only compute the causally valid column range, and
only the diagonal 128-wide sub-block needs a mask multiply (on GpSimd, which
is otherwise idle).

All matmul operands are fp16 (same 1 cyc/row PE rate as fp32r at N>=256 but
no 4x penalty at N=128, and half the DMA/SBUF traffic); PSUM accumulation and
the softmax normalization stay f32. Phase B is software-pipelined per k-tile:
QK+exp issue ahead, AV lags LAG tiles so the in-order PE queue never waits on
the Scalar-engine exp; the normalization units of each head-pair group are
displaced into the next group's tile stream for the same reason.
"""

import os
import sys

import numpy as np

for _p in ("/opt/trn_rl_repo", "/root/.axon_site/_ro/trn_rl_repo"):
    if os.path.isdir(_p) and _p not in sys.path:
        sys.path.append(_p)

import concourse.bass as bass  # noqa: E402,F401
import concourse.mybir as mybir  # noqa: E402
import concourse.tile as tile  # noqa: E402
from concourse import bacc, bass_utils  # noqa: E402

f32 = mybir.dt.float32
f32r = mybir.dt.float32r
F16 = mybir.dt.float16

B, T, C = 4, 2048, 1024
N_HEAD, D = 16, 64
NCORES = 8
HPC = 8  # heads per core
CH = HPC * D  # 512 channels per core
P = 128
NQ = 512  # q-strip width
NSTRIP = T // NQ  # 4
SCALE = 1.0 / 8.0  # 1/sqrt(D)
LAG = 5  # AV trails QK/exp by this many k-tiles in the PE stream

MM_DT = F16


def build():
    nc = bacc.Bacc("TRN2", target_bir_lowering=False, debug=False)
    xt = nc.dram_tensor("xt", (C, T), MM_DT, kind="ExternalInput")
    wq = nc.dram_tensor("wq", (C, CH), MM_DT, kind="ExternalInput")
    wk = nc.dram_tensor("wk", (C, CH), MM_DT, kind="ExternalInput")
    wv = nc.dram_tensor("wv", (C, CH), MM_DT, kind="ExternalInput")
    wp = nc.dram_tensor("wp", (CH, C), MM_DT, kind="ExternalInput")
    # mk: cols 0-127 identity, cols 128-255 additive causal mask
    # (0 keep / -60000 drop)
    mk = nc.dram_tensor("mk", (P, 256), F16, kind="ExternalInput")
    out = nc.dram_tensor("out", (T, C), F16, kind="ExternalOutput")
    Exp = mybir.ActivationFunctionType.Exp

    with tile.TileContext(nc) as tc:
        with (
            tc.tile_pool(name="sb", bufs=1) as sb,
            tc.tile_pool(name="ps", bufs=1, space="PSUM") as psp,
        ):
            mask = sb.tile([P, 256], F16, tag="mask", bufs=1, name="mask")
            nc.sync.dma_start(mask[:], mk[:])
            ident = mask[:, 0:128]
            maskneg = mask[:, 128:256]
            # PE warmup: stream zeros through the PE while the weight/x DMAs
            # land so the tensor engine p-state ramps before real work.
            # (memset first so the warmup isn't gated on the other memsets)
            wrm = sb.tile([P, NQ], F16, tag="wrm", bufs=1, name="wrm")
            nc.vector.memset(wrm[:], 0.0)
            for i in range(16):
                wps_ = psp.tile([P, NQ], f32, tag="mm", bufs=2, name="wrmp")
                nc.tensor.matmul(wps_[:], wrm[:, 0:P], wrm[:], start=True, stop=True)
            sel = sb.tile([33, P], F16, tag="sel", bufs=1, name="sel")
            nc.vector.memset(sel[:], 0.0)
            nc.vector.memset(sel[0:1, 0:64], 1.0)
            nc.vector.memset(sel[32:33, 64:128], 1.0)
            col1 = sb.tile([P, HPC], F16, tag="ones8", bufs=1, name="col1")
            nc.vector.memset(col1[:], 1.0)
            # den rows 1..31 must be a harmless non-zero constant for the
            # reciprocal (sel zeroes them in the broadcast); memset once,
            # rows 0/32 are overwritten per group.
            dens = [
                sb.tile([33, NQ], f32, tag="den", bufs=2, name=f"den{i}")
                for i in range(2)
            ]
            for dt_ in dens:
                nc.vector.memset(dt_[:], 1.0)

            # ---- weight + first-strip x loads, interleaved so the first QT
            # accumulation chain unblocks after two small DMAs. x issues on
            # the (startup-idle) Activation engine in parallel with Sync.
            wq_sb, wk_sb, wv_sb = ([] for _ in range(3))
            xts = {}  # strip -> list of 8 c-tiles
            xdma_done = set()

            def x_dma(s, eng):
                if s in xdma_done or s >= NSTRIP:
                    return
                xdma_done.add(s)
                xts[s] = []
                for c in range(8):
                    t = sb.tile([P, NQ], MM_DT, tag="x", bufs=24, name=f"x{s}_{c}")
                    eng.dma_start(t[:], xt[c * P : (c + 1) * P, s * NQ : (s + 1) * NQ])
                    xts[s].append(t)

            for c in range(8):
                t = sb.tile([P, CH], MM_DT, tag="w", bufs=32, name=f"wq{c}")
                nc.sync.dma_start(t[:], wq[c * P : (c + 1) * P, :])
                wq_sb.append(t)
            x_dma(0, nc.scalar)
            for dram, lst, nm in ((wk, wk_sb, "wk"), (wv, wv_sb, "wv")):
                for c in range(8):
                    t = sb.tile([P, CH], MM_DT, tag="w", bufs=32, name=f"{nm}{c}")
                    nc.sync.dma_start(t[:], dram[c * P : (c + 1) * P, :])
                    lst.append(t)
            wp_sb = []
            for c in range(4):
                for n in range(2):
                    t = sb.tile([P, NQ], MM_DT, tag="w", bufs=32, name=f"wp{c}{n}")
                    nc.scalar.dma_start(
                        t[:], wp[c * P : (c + 1) * P, n * NQ : (n + 1) * NQ]
                    )
                    wp_sb.append(t)

            kts = [sb.tile([P, T], MM_DT, tag="kt", bufs=4, name=f"kt{m}") for m in range(4)]
            vts = [None] * 16
            qts = {}

            # ---- Phase A: QT (strip-local), KT (transposed) and V (ones-augmented)
            def a_units(s):
                def u_dma():
                    x_dma(s, nc.sync)
                    x_dma(s + 1, nc.sync)
                    qts[s] = []

                def u_qt(m):
                    ps = psp.tile([P, NQ], f32, tag="mm", bufs=2, name="psa")
                    for c in range(8):
                        nc.tensor.matmul(
                            ps[:],
                            wq_sb[c][:, m * P : (m + 1) * P],
                            xts[s][c][:],
                            start=(c == 0),
                            stop=(c == 7),
                        )
                    qt_t = sb.tile([P, NQ], MM_DT, tag="qt", bufs=8, name=f"q{s}_{m}")
                    nc.vector.tensor_copy(qt_t[:], ps[:])
                    qts[s].append(qt_t)

                def u_kt(m):
                    ps = psp.tile([P, NQ], f32, tag="mm", bufs=2, name="psk")
                    for c in range(8):
                        nc.tensor.matmul(
                            ps[:],
                            wk_sb[c][:, m * P : (m + 1) * P],
                            xts[s][c][:],
                            start=(c == 0),
                            stop=(c == 7),
                        )
                    nc.vector.tensor_copy(kts[m][:, s * NQ : (s + 1) * NQ], ps[:])

                def u_v(mt):
                    g = s * 4 + mt
                    ps = psp.tile([P, NQ], f32, tag="mm", bufs=2, name="psv")
                    for c in range(8):
                        nc.tensor.matmul(
                            ps[:],
                            xts[s][c][:, mt * P : (mt + 1) * P],
                            wv_sb[c][:],
                            start=(c == 0),
                            stop=(c == 7),
                        )
                    vt = sb.tile([P, HPC * 65], F16, tag="v", bufs=16, name=f"v{g}")
                    v3 = vt.rearrange("p (h e) -> p h e", e=65)
                    # V copy must stay off ACT: queued ahead of ready exps it
                    # head-of-line blocks the attention pipeline.
                    nc.vector.tensor_copy(v3[:, :, 0:64], ps.rearrange("p (h e) -> p h e", e=64))
                    nc.vector.tensor_copy(
                        v3[:, :, 64:65], col1[:].rearrange("p (h e) -> p h e", e=1)
                    )
                    vts[g] = vt

                units = [u_dma]
                for m in range(4):
                    units.append(lambda m=m: u_qt(m))
                    units.append(lambda m=m: u_kt(m))
                    units.append(lambda m=m: u_v(m))
                return units

            # ---- Phase B: flash attention, software-pipelined per k-tile.
            yts = {}
            pending_norms = []  # norm units of the previous head-pair group

            def bc_units(s, c):
                nkt = 4 * (s + 1)
                st = {}

                def u_tile(kt):
                    j = kt - 4 * s  # >=0: diagonal-straddling tile
                    off = 128 * max(j, 0)
                    qkp = psp.tile([P, 2 * NQ], f32, tag="qk", bufs=2, name="qkp")
                    for sub in range(2):
                        nc.tensor.matmul(
                            qkp[:, sub * NQ + off : (sub + 1) * NQ],
                            kts[c][sub * 64 : (sub + 1) * 64, kt * P : (kt + 1) * P],
                            qts[s][c][sub * 64 : (sub + 1) * 64, off:NQ],
                            start=True,
                            stop=True,
                        )
                        if j >= 0:
                            # causal mask for the diagonal 128-block folded in
                            # as a PE accumulation: scores += -60000 above the
                            # diagonal, so exp() masks for free and the
                            # exp->AV chain has no extra engine hop.
                            nc.tensor.matmul(
                                qkp[:, sub * NQ + off : sub * NQ + off + 128],
                                ident,
                                maskneg,
                                start=False,
                                stop=True,
                                skip_group_check=True,
                            )
                    att = sb.tile([P, 2 * NQ], F16, tag="att", bufs=LAG + 7, name="att")
                    nc.scalar.activation(
                        att.rearrange("p (u q) -> p u q", u=2)[:, :, off:NQ],
                        qkp.rearrange("p (u q) -> p u q", u=2)[:, :, off:NQ],
                        Exp,
                        scale=SCALE,
                    )
                    st[kt] = (att, off)

                def u_av(kt):
                    if "av" not in st:
                        st["av"] = [
                            psp.tile([65, NQ], f32, tag="sm", bufs=2, name=f"av{s}{c}{u}")
                            for u in range(2)
                        ]
                    att, off = st.pop(kt)
                    for sub in range(2):
                        h = 2 * c + sub
                        nc.tensor.matmul(
                            st["av"][sub][:, off:NQ],
                            vts[kt][:, h * 65 : (h + 1) * 65],
                            att[:, sub * NQ + off : (sub + 1) * NQ],
                            start=(kt == 0),
                            stop=(kt == nkt - 1),
                        )

                def u_norm_a():
                    # denominators -> reciprocals (DVE), cast f16 for the
                    # 1 cyc/row selector matmul
                    av = st["av"]
                    den = dens[(4 * s + c) % 2]
                    rcf = sb.tile([33, NQ], f32, tag="rcf", bufs=2, name="rcf")
                    rc2 = sb.tile([33, NQ], F16, tag="rc2", bufs=2, name="rc2")
                    for sub in range(2):
                        nc.vector.tensor_copy(
                            den[32 * sub : 32 * sub + 1, :], av[sub][64:65, :]
                        )
                    nc.vector.reciprocal_approx_fast(out=rcf[:], in_=den[:])
                    nc.vector.tensor_copy(rc2[:], rcf[:])
                    st["rc2"] = rc2

                def u_norm_b():
                    # broadcast reciprocals to 128 partitions via selector matmul
                    bc_ps = psp.tile([P, NQ], f32, tag="mm", bufs=2, name="bcp")
                    nc.tensor.matmul(bc_ps[:], sel[:], st["rc2"][:], start=True, stop=True)
                    st["bc_ps"] = bc_ps

                def u_norm_c():
                    bc = sb.tile([P, NQ], f32, tag="bc", bufs=2, name="bc")
                    nc.vector.tensor_copy(bc[:], st["bc_ps"][:])
                    av = st["av"]
                    yts[(c, s)] = sb.tile([P, NQ], MM_DT, tag="yt", bufs=16, name=f"y{c}{s}")
                    for sub in range(2):
                        nc.vector.tensor_mul(
                            yts[(c, s)][sub * 64 : (sub + 1) * 64, :],
                            av[sub][0:64, :],
                            bc[sub * 64 : (sub + 1) * 64, :],
                        )

                units = []
                for kt in range(nkt):
                    def u(kt=kt):
                        u_tile(kt)
                        if kt - LAG >= 0:
                            u_av(kt - LAG)
                    units.append(u)
                # AV tail (exp-gated) + norm chain are displaced into the
                # next group's tile stream: its early QKs are independent of
                # this group's last exps, so the PE never idles on them.
                tail = [lambda kt=kt: u_av(kt) for kt in range(max(nkt - LAG, 0), nkt)]
                tail += [u_norm_a, u_norm_b, u_norm_c]
                return units, tail

            def b_units(s):
                pending = pending_norms[:]
                units = []
                for c in range(4):
                    tiles, tail = bc_units(s, c)
                    merged = []
                    for i, u in enumerate(tiles):
                        merged.append(u)
                        if i < len(pending):
                            merged.append(pending[i])
                    if len(pending) > len(tiles):
                        merged.extend(pending[len(tiles):])
                    units.extend(merged)
                    pending = tail
                pending_norms[:] = pending
                return units

            # ---- Phase C: partial projection (host reduces across head groups)
            def c_units(s):
                def u_proj(o, n):
                    m = 4 * s + o
                    ps = psp.tile([P, NQ], f32, tag="mm", bufs=2, name="psc")
                    for c in range(4):
                        nc.tensor.matmul(
                            ps[:],
                            yts[(c, s)][:, o * P : (o + 1) * P],
                            wp_sb[c * 2 + n][:],
                            start=(c == 0),
                            stop=(c == 3),
                        )
                    ot = sb.tile([P, NQ], F16, tag="ot", bufs=3, name="ot")
                    nc.vector.tensor_copy(ot[:], ps[:])
                    nc.sync.dma_start(out[m * P : (m + 1) * P, n * NQ : (n + 1) * NQ], ot[:])

                return [lambda o=o, n=n: u_proj(o, n) for o in range(4) for n in range(2)]

            # ---- Driver: emit B(s) woven with C(s-1) then A(s+1) so the PE
            # queue alternates attention work with projection/qkv matmuls
            # (which fill PE while ACT runs the exps).
            def weave(primary, secondary, hold=9):
                # hold: emit this many primary units before the first
                # secondary one (the strip-crossing displaced norm units sit
                # in the first few primary slots and C units depend on them)
                np_, ns_ = len(primary), len(secondary)
                hold = min(hold, np_ - 1)
                emitted = 0
                for i, u in enumerate(primary):
                    u()
                    want = max(0, (i + 1 - hold)) * ns_ // (np_ - hold)
                    while emitted < want:
                        secondary[emitted]()
                        emitted += 1
                while emitted < ns_:
                    secondary[emitted]()
                    emitted += 1

            # C(s) is woven two strips later (C0->B2, C1/C2->B3): late strips
            # are locally ACT(exp)-bound, so they need the projection matmuls
            # as PE filler; early strips are PE-bound and don't.
            for u in a_units(0):
                u()
            for s in range(NSTRIP):
                others = []
                if s + 1 < NSTRIP:
                    others.extend(a_units(s + 1))
                if s == 2:
                    others.extend(c_units(0))
                elif s == 3:
                    others.extend(c_units(1))
                    others.extend(c_units(2))
                weave(b_units(s), others)
            for u in pending_norms:
                u()
            for u in c_units(NSTRIP - 1):
                u()

    nc.compile()
    return nc


_NC = None


def _get_nc():
    global _NC
    if _NC is None:
        _NC = build()
    return _NC


def host_mask():
    # cols 0-127: identity; cols 128-255: additive causal mask for a
    # diagonal 128x128 block (0 where k <= q, -60000 where k > q)
    m = np.zeros((P, 256), np.float16)
    m[:, 0:128] = np.eye(P, dtype=np.float16)
    for kk in range(P):
        m[kk, 128 : 128 + kk] = -60000.0
    return m


def make_in_maps(x, w_qkv, w_proj):
    x = np.asarray(x, np.float32)
    w_qkv = np.asarray(w_qkv, np.float16)
    w_proj = np.asarray(w_proj, np.float16)
    mkm = host_mask()
    in_maps = []
    for core in range(NCORES):
        b, hg = core // 2, core % 2
        lo, hi = hg * CH, (hg + 1) * CH
        in_maps.append(
            {
                "xt": np.ascontiguousarray(x[b].T.astype(np.float16)),
                "wq": np.ascontiguousarray(w_qkv[:, lo:hi]),
                "wk": np.ascontiguousarray(w_qkv[:, C + lo : C + hi]),
                "wv": np.ascontiguousarray(w_qkv[:, 2 * C + lo : 2 * C + hi]),
                "wp": np.ascontiguousarray(w_proj[lo:hi, :]),
                "mk": mkm,
            }
        )
    return in_maps


def kernel(x, w_qkv, w_proj):
    in_maps = make_in_maps(x, w_qkv, w_proj)
    last_err = None
    for attempt in range(3):
        try:
            res = bass_utils.run_bass_kernel_spmd(
                _get_nc(), in_maps, core_ids=list(range(NCORES))
            )
            break
        except Exception as e:  # transient device wedge: back off and retry
            last_err = e
            import time

            time.sleep(10 * (attempt + 1))
    else:
        raise last_err
    out = np.empty((B, T, C), np.float32)
    for b in range(B):
        out[b] = res.results[2 * b]["out"].astype(np.float32) + res.results[
            2 * b + 1
        ]["out"].astype(np.float32)
    return out


# revision 23
# speedup vs baseline: 1.0806x; 1.0806x over previous
"""Causal self-attention (B=4, T=2048, C=1024, 16 heads) on 8 TRN2 NeuronCores.

Sharding: data-parallel over batch (4) x tensor-parallel over heads (2 groups
of 8). Each core computes qkv + attention for its 8 heads and a partial
output projection (row-parallel); the host sums the two partials per batch.

Everything on-chip runs in a transposed layout so no tensor ever needs an
on-device transpose:
  QT/KT [ch, t]  <- W^T @ x^T      (x^T prepared on host)
  attT  [k, q]   = exp(K @ Q^T / 8) * causal_mask
  yT    [ch, q]  = V_aug^T @ attT  (V augmented with a ones column per head ->
                                    row 64 of each head's block = softmax denom)
  out   [q, c]   = yT^T @ Wp       (partial; host-reduced across head groups)

Softmax skips max-subtraction (scores/8 are O(1) here, exp is safe), which is
mathematically identical to the reference; fully-masked blocks are never
computed, straddle blocks only compute the causally valid column range, and
only the diagonal 128-wide sub-block needs a mask multiply (on GpSimd, which
is otherwise idle).

All matmul operands are fp16 (same 1 cyc/row PE rate as fp32r at N>=256 but
no 4x penalty at N=128, and half the DMA/SBUF traffic); PSUM accumulation and
the softmax normalization stay f32. Phase B is software-pipelined per k-tile:
QK+exp issue ahead, AV lags LAG tiles so the in-order PE queue never waits on
the Scalar-engine exp; the normalization units of each head-pair group are
displaced into the next group's tile stream for the same reason.
"""

import os
import sys

import numpy as np

for _p in ("/opt/trn_rl_repo", "/root/.axon_site/_ro/trn_rl_repo"):
    if os.path.isdir(_p) and _p not in sys.path:
        sys.path.append(_p)

import concourse.bass as bass  # noqa: E402,F401
import concourse.mybir as mybir  # noqa: E402
import concourse.tile as tile  # noqa: E402
from concourse import bacc, bass_utils  # noqa: E402

f32 = mybir.dt.float32
f32r = mybir.dt.float32r
F16 = mybir.dt.float16

B, T, C = 4, 2048, 1024
N_HEAD, D = 16, 64
NCORES = 8
HPC = 8  # heads per core
CH = HPC * D  # 512 channels per core
P = 128
NQ = 512  # q-strip width
NSTRIP = T // NQ  # 4
SCALE = 1.0 / 8.0  # 1/sqrt(D)
LAG = 5  # AV trails QK/exp by this many k-tiles in the PE stream

MM_DT = F16


def build():
    nc = bacc.Bacc("TRN2", target_bir_lowering=False, debug=False)
    xt = nc.dram_tensor("xt", (C, T), MM_DT, kind="ExternalInput")
    wq = nc.dram_tensor("wq", (C, CH), MM_DT, kind="ExternalInput")
    wk = nc.dram_tensor("wk", (C, CH), MM_DT, kind="ExternalInput")
    wv = nc.dram_tensor("wv", (C, CH), MM_DT, kind="ExternalInput")
    wp = nc.dram_tensor("wp", (CH, C), MM_DT, kind="ExternalInput")
    # mk: cols 0-127 identity, cols 128-255 additive causal mask
    # (0 keep / -60000 drop)
    mk = nc.dram_tensor("mk", (P, 256), F16, kind="ExternalInput")
    out = nc.dram_tensor("out", (T, C), F16, kind="ExternalOutput")
    Exp = mybir.ActivationFunctionType.Exp

    with tile.TileContext(nc) as tc:
        with (
            tc.tile_pool(name="sb", bufs=1) as sb,
            tc.tile_pool(name="ps", bufs=1, space="PSUM") as psp,
        ):
            mask = sb.tile([P, 256], F16, tag="mask", bufs=1, name="mask")
            nc.sync.dma_start(mask[:], mk[:])
            maskmul = mask[:, 0:128]
            # PE warmup: stream zeros through the PE while the weight/x DMAs
            # land so the tensor engine p-state ramps before real work.
            # (memset first so the warmup isn't gated on the other memsets)
            wrm = sb.tile([P, NQ], F16, tag="wrm", bufs=1, name="wrm")
            nc.vector.memset(wrm[:], 0.0)
            for i in range(16):
                wps_ = psp.tile([P, NQ], f32, tag="mm", bufs=2, name="wrmp")
                nc.tensor.matmul(wps_[:], wrm[:, 0:P], wrm[:], start=True, stop=True)
            sel = sb.tile([33, P], F16, tag="sel", bufs=1, name="sel")
            nc.vector.memset(sel[:], 0.0)
            nc.vector.memset(sel[0:1, 0:64], 1.0)
            nc.vector.memset(sel[32:33, 64:128], 1.0)
            col1 = sb.tile([P, HPC], F16, tag="ones8", bufs=1, name="col1")
            nc.vector.memset(col1[:], 1.0)
            # den rows 1..31 must be a harmless non-zero constant for the
            # reciprocal (sel zeroes them in the broadcast); memset once,
            # rows 0/32 are overwritten per group.
            dens = [
                sb.tile([33, NQ], f32, tag="den", bufs=2, name=f"den{i}")
                for i in range(2)
            ]
            for dt_ in dens:
                nc.vector.memset(dt_[:], 1.0)

            # ---- weight + first-strip x loads, interleaved so the first QT
            # accumulation chain unblocks after two small DMAs. x issues on
            # the (startup-idle) Activation engine in parallel with Sync.
            wq_sb, wk_sb, wv_sb = ([] for _ in range(3))
            xts = {}  # strip -> list of 8 c-tiles
            xdma_done = set()

            def x_dma(s, eng):
                if s in xdma_done or s >= NSTRIP:
                    return
                xdma_done.add(s)
                xts[s] = []
                for c in range(8):
                    t = sb.tile([P, NQ], MM_DT, tag="x", bufs=24, name=f"x{s}_{c}")
                    eng.dma_start(t[:], xt[c * P : (c + 1) * P, s * NQ : (s + 1) * NQ])
                    xts[s].append(t)

            for c in range(8):
                t = sb.tile([P, CH], MM_DT, tag="w", bufs=32, name=f"wq{c}")
                nc.sync.dma_start(t[:], wq[c * P : (c + 1) * P, :])
                wq_sb.append(t)
            x_dma(0, nc.scalar)
            for dram, lst, nm in ((wk, wk_sb, "wk"), (wv, wv_sb, "wv")):
                for c in range(8):
                    t = sb.tile([P, CH], MM_DT, tag="w", bufs=32, name=f"{nm}{c}")
                    nc.sync.dma_start(t[:], dram[c * P : (c + 1) * P, :])
                    lst.append(t)
            wp_sb = []
            for c in range(4):
                for n in range(2):
                    t = sb.tile([P, NQ], MM_DT, tag="w", bufs=32, name=f"wp{c}{n}")
                    nc.scalar.dma_start(
                        t[:], wp[c * P : (c + 1) * P, n * NQ : (n + 1) * NQ]
                    )
                    wp_sb.append(t)

            kts = [sb.tile([P, T], MM_DT, tag="kt", bufs=4, name=f"kt{m}") for m in range(4)]
            vts = [None] * 16
            qts = {}

            # ---- Phase A: QT (strip-local), KT (transposed) and V (ones-augmented)
            def a_units(s):
                def u_dma():
                    x_dma(s, nc.sync)
                    x_dma(s + 1, nc.sync)
                    qts[s] = []

                def u_qt(m):
                    ps = psp.tile([P, NQ], f32, tag="mm", bufs=2, name="psa")
                    for c in range(8):
                        nc.tensor.matmul(
                            ps[:],
                            wq_sb[c][:, m * P : (m + 1) * P],
                            xts[s][c][:],
                            start=(c == 0),
                            stop=(c == 7),
                        )
                    qt_t = sb.tile([P, NQ], MM_DT, tag="qt", bufs=8, name=f"q{s}_{m}")
                    nc.vector.tensor_copy(qt_t[:], ps[:])
                    qts[s].append(qt_t)

                def u_kt(m):
                    ps = psp.tile([P, NQ], f32, tag="mm", bufs=2, name="psk")
                    for c in range(8):
                        nc.tensor.matmul(
                            ps[:],
                            wk_sb[c][:, m * P : (m + 1) * P],
                            xts[s][c][:],
                            start=(c == 0),
                            stop=(c == 7),
                        )
                    nc.vector.tensor_copy(kts[m][:, s * NQ : (s + 1) * NQ], ps[:])

                def u_v(mt):
                    g = s * 4 + mt
                    ps = psp.tile([P, NQ], f32, tag="mm", bufs=2, name="psv")
                    for c in range(8):
                        nc.tensor.matmul(
                            ps[:],
                            xts[s][c][:, mt * P : (mt + 1) * P],
                            wv_sb[c][:],
                            start=(c == 0),
                            stop=(c == 7),
                        )
                    vt = sb.tile([P, HPC * 65], F16, tag="v", bufs=16, name=f"v{g}")
                    v3 = vt.rearrange("p (h e) -> p h e", e=65)
                    # V copy must stay off ACT: queued ahead of ready exps it
                    # head-of-line blocks the attention pipeline.
                    nc.vector.tensor_copy(v3[:, :, 0:64], ps.rearrange("p (h e) -> p h e", e=64))
                    nc.vector.tensor_copy(
                        v3[:, :, 64:65], col1[:].rearrange("p (h e) -> p h e", e=1)
                    )
                    vts[g] = vt

                units = [u_dma]
                for m in range(4):
                    units.append(lambda m=m: u_qt(m))
                    units.append(lambda m=m: u_kt(m))
                    units.append(lambda m=m: u_v(m))
                return units

            # ---- Phase B: flash attention, software-pipelined per k-tile.
            yts = {}
            pending_norms = []  # norm units of the previous head-pair group

            def bc_units(s, c):
                nkt = 4 * (s + 1)
                st = {}

                def u_tile(kt):
                    j = kt - 4 * s  # >=0: diagonal-straddling tile
                    off = 128 * max(j, 0)
                    qkp = psp.tile([P, 2 * NQ], f32, tag="qk", bufs=2, name="qkp")
                    for sub in range(2):
                        nc.tensor.matmul(
                            qkp[:, sub * NQ + off : (sub + 1) * NQ],
                            kts[c][sub * 64 : (sub + 1) * 64, kt * P : (kt + 1) * P],
                            qts[s][c][sub * 64 : (sub + 1) * 64, off:NQ],
                            start=True,
                            stop=True,
                        )
                    att = sb.tile([P, 2 * NQ], F16, tag="att", bufs=LAG + 7, name="att")
                    nc.scalar.activation(
                        att.rearrange("p (u q) -> p u q", u=2)[:, :, off:NQ],
                        qkp.rearrange("p (u q) -> p u q", u=2)[:, :, off:NQ],
                        Exp,
                        scale=SCALE,
                    )
                    if j >= 0:
                        for sub in range(2):
                            nc.gpsimd.tensor_mul(
                                att[:, sub * NQ + off : sub * NQ + off + 128],
                                att[:, sub * NQ + off : sub * NQ + off + 128],
                                maskmul,
                            )
                    st[kt] = (att, off)

                def u_av(kt):
                    if "av" not in st:
                        st["av"] = [
                            psp.tile([65, NQ], f32, tag="sm", bufs=2, name=f"av{s}{c}{u}")
                            for u in range(2)
                        ]
                    att, off = st.pop(kt)
                    for sub in range(2):
                        h = 2 * c + sub
                        nc.tensor.matmul(
                            st["av"][sub][:, off:NQ],
                            vts[kt][:, h * 65 : (h + 1) * 65],
                            att[:, sub * NQ + off : (sub + 1) * NQ],
                            start=(kt == 0),
                            stop=(kt == nkt - 1),
                        )

                def u_norm_a():
                    # denominators -> reciprocals (DVE), cast f16 for the
                    # 1 cyc/row selector matmul
                    av = st["av"]
                    den = dens[(4 * s + c) % 2]
                    rcf = sb.tile([33, NQ], f32, tag="rcf", bufs=2, name="rcf")
                    rc2 = sb.tile([33, NQ], F16, tag="rc2", bufs=2, name="rc2")
                    for sub in range(2):
                        nc.vector.tensor_copy(
                            den[32 * sub : 32 * sub + 1, :], av[sub][64:65, :]
                        )
                    nc.vector.reciprocal_approx_fast(out=rcf[:], in_=den[:])
                    nc.vector.tensor_copy(rc2[:], rcf[:])
                    st["rc2"] = rc2

                def u_norm_b():
                    # broadcast reciprocals to 128 partitions via selector matmul
                    bc_ps = psp.tile([P, NQ], f32, tag="mm", bufs=2, name="bcp")
                    nc.tensor.matmul(bc_ps[:], sel[:], st["rc2"][:], start=True, stop=True)
                    st["bc_ps"] = bc_ps

                def u_norm_c():
                    bc = sb.tile([P, NQ], f32, tag="bc", bufs=2, name="bc")
                    nc.vector.tensor_copy(bc[:], st["bc_ps"][:])
                    av = st["av"]
                    yts[(c, s)] = sb.tile([P, NQ], MM_DT, tag="yt", bufs=16, name=f"y{c}{s}")
                    for sub in range(2):
                        nc.vector.tensor_mul(
                            yts[(c, s)][sub * 64 : (sub + 1) * 64, :],
                            av[sub][0:64, :],
                            bc[sub * 64 : (sub + 1) * 64, :],
                        )

                units = []
                for kt in range(nkt):
                    def u(kt=kt):
                        u_tile(kt)
                        if kt - LAG >= 0:
                            u_av(kt - LAG)
                    units.append(u)
                # AV tail (exp-gated) + norm chain are displaced into the
                # next group's tile stream: its early QKs are independent of
                # this group's last exps, so the PE never idles on them.
                tail = [lambda kt=kt: u_av(kt) for kt in range(max(nkt - LAG, 0), nkt)]
                tail += [u_norm_a, u_norm_b, u_norm_c]
                return units, tail

            def b_units(s):
                pending = pending_norms[:]
                units = []
                for c in range(4):
                    tiles, tail = bc_units(s, c)
                    merged = []
                    for i, u in enumerate(tiles):
                        merged.append(u)
                        if i < len(pending):
                            merged.append(pending[i])
                    if len(pending) > len(tiles):
                        merged.extend(pending[len(tiles):])
                    units.extend(merged)
                    pending = tail
                pending_norms[:] = pending
                return units

            # ---- Phase C: partial projection (host reduces across head groups)
            def c_units(s):
                def u_proj(o, n):
                    m = 4 * s + o
                    ps = psp.tile([P, NQ], f32, tag="mm", bufs=2, name="psc")
                    for c in range(4):
                        nc.tensor.matmul(
                            ps[:],
                            yts[(c, s)][:, o * P : (o + 1) * P],
                            wp_sb[c * 2 + n][:],
                            start=(c == 0),
                            stop=(c == 3),
                        )
                    ot = sb.tile([P, NQ], F16, tag="ot", bufs=3, name="ot")
                    nc.vector.tensor_copy(ot[:], ps[:])
                    nc.sync.dma_start(out[m * P : (m + 1) * P, n * NQ : (n + 1) * NQ], ot[:])

                return [lambda o=o, n=n: u_proj(o, n) for o in range(4) for n in range(2)]

            # ---- Driver: emit B(s) woven with C(s-1) then A(s+1) so the PE
            # queue alternates attention work with projection/qkv matmuls
            # (which fill PE while ACT runs the exps).
            def weave(primary, secondary, hold=9):
                # hold: emit this many primary units before the first
                # secondary one (the strip-crossing displaced norm units sit
                # in the first few primary slots and C units depend on them)
                np_, ns_ = len(primary), len(secondary)
                hold = min(hold, np_ - 1)
                emitted = 0
                for i, u in enumerate(primary):
                    u()
                    want = max(0, (i + 1 - hold)) * ns_ // (np_ - hold)
                    while emitted < want:
                        secondary[emitted]()
                        emitted += 1
                while emitted < ns_:
                    secondary[emitted]()
                    emitted += 1

            # C(s) is woven two strips later (C0->B2, C1/C2->B3): late strips
            # are locally ACT(exp)-bound, so they need the projection matmuls
            # as PE filler; early strips are PE-bound and don't.
            for u in a_units(0):
                u()
            for s in range(NSTRIP):
                others = []
                if s + 1 < NSTRIP:
                    others.extend(a_units(s + 1))
                if s == 2:
                    others.extend(c_units(0))
                elif s == 3:
                    others.extend(c_units(1))
                    others.extend(c_units(2))
                weave(b_units(s), others)
            for u in pending_norms:
                u()
            for u in c_units(NSTRIP - 1):
                u()

    nc.compile()
    return nc


_NC = None


def _get_nc():
    global _NC
    if _NC is None:
        _NC = build()
    return _NC


def host_mask():
    # cols 0-127: multiplicative causal mask for a diagonal 128x128 block
    # (keep k <= q); cols 128-255 unused
    m = np.zeros((P, 256), np.float16)
    for kk in range(P):
        m[kk, kk:128] = 1.0
    return m


def make_in_maps(x, w_qkv, w_proj):
    x = np.asarray(x, np.float32)
    w_qkv = np.asarray(w_qkv, np.float16)
    w_proj = np.asarray(w_proj, np.float16)
    mkm = host_mask()
    in_maps = []
    for core in range(NCORES):
        b, hg = core // 2, core % 2
        lo, hi = hg * CH, (hg + 1) * CH
        in_maps.append(
            {
                "xt": np.ascontiguousarray(x[b].T.astype(np.float16)),
                "wq": np.ascontiguousarray(w_qkv[:, lo:hi]),
                "wk": np.ascontiguousarray(w_qkv[:, C + lo : C + hi]),
                "wv": np.ascontiguousarray(w_qkv[:, 2 * C + lo : 2 * C + hi]),
                "wp": np.ascontiguousarray(w_proj[lo:hi, :]),
                "mk": mkm,
            }
        )
    return in_maps


def kernel(x, w_qkv, w_proj):
    in_maps = make_in_maps(x, w_qkv, w_proj)
    last_err = None
    for attempt in range(3):
        try:
            res = bass_utils.run_bass_kernel_spmd(
                _get_nc(), in_maps, core_ids=list(range(NCORES))
            )
            break
        except Exception as e:  # transient device wedge: back off and retry
            last_err = e
            import time

            time.sleep(10 * (attempt + 1))
    else:
        raise last_err
    out = np.empty((B, T, C), np.float32)
    for b in range(B):
        out[b] = res.results[2 * b]["out"].astype(np.float32) + res.results[
            2 * b + 1
        ]["out"].astype(np.float32)
    return out
